# revision 1
# baseline (speedup 1.0000x reference)
"""CCNF RK4 sampling kernel for 8 Trainium2 NeuronCores.

Data-parallel: batch 2048 -> 256 per core, weights replicated.
On-core layout: features on partitions, batch on the free dim (N=256).
Matmuls run in float32r (fp32 data, fast PE mode).
"""

import os

import numpy as np

N_CORES = 8


def _build_program(theta0, context, W1, b1, W2, b2, W3, b3, n_steps):
    import concourse.bass as bass
    import concourse.mybir as mybir
    import concourse.tile as tile
    from concourse import bacc
    from concourse.bass_utils import run_bass_kernel_spmd

    f32 = mybir.dt.float32
    f32r = mybir.dt.float32r
    ALU = mybir.AluOpType
    SIGMOID = mybir.ActivationFunctionType.Sigmoid

    B, D = theta0.shape          # 2048, 32
    C = context.shape[1]         # 128
    IN, H2 = W1.shape            # 161, 1024
    H = W2.shape[0]              # 512
    assert H2 == 2 * H and W2.shape[1] == 2 * H and W3.shape == (H, D)
    assert IN == D + 1 + C
    assert B % N_CORES == 0
    Bs = B // N_CORES            # 256 per core
    steps = int(n_steps)
    dt = 1.0 / steps

    KC = H // 128                # 4 k-chunks for layer 2/3
    MJ = H // 128                # 4 output column-chunks per GLU half
    # layer-1 K split: rows [0:33) = theta(32)+t(1) (dynamic); rows [33:161) = ctx (static)
    K1A = D + 1                  # 33 (theta + t)
    K1B = IN - K1A               # 128 (ctx)

    # ---- host-side layout prep (shared across cores) ----
    W1 = np.ascontiguousarray(W1, np.float32)
    w1c1_h = np.ascontiguousarray(W1[:K1A])                    # [33, 1024]  theta+t rows
    w1c2_h = np.ascontiguousarray(W1[K1A:])                    # [128, 1024] ctx rows
    b3_is_zero = not np.any(np.asarray(b3, np.float32))
    # [512,1024] -> [128, 4*1024]: chunk kc at cols kc*1024
    w2_h = np.ascontiguousarray(
        np.asarray(W2, np.float32).reshape(KC, 128, 2 * H).transpose(1, 0, 2).reshape(128, KC * 2 * H)
    )
    # [512,32] -> [128, 4*32]
    w3_h = np.ascontiguousarray(
        np.asarray(W3, np.float32).reshape(KC, 128, D).transpose(1, 0, 2).reshape(128, KC * D)
    )
    b1 = np.asarray(b1, np.float32)
    b2 = np.asarray(b2, np.float32)
    bias_h = np.ascontiguousarray(np.concatenate([
        b1[:H].reshape(MJ, 128).T, b1[H:].reshape(MJ, 128).T,
        b2[:H].reshape(MJ, 128).T, b2[H:].reshape(MJ, 128).T,
    ], axis=1))                                                # [128, 16]
    onesb3_h = np.ascontiguousarray(np.concatenate([
        np.ones((1, Bs), np.float32),
        np.asarray(b3, np.float32).reshape(1, D),
    ], axis=1))                                                # [1, Bs+32]

    # ---- build the bass program (same program on all 8 cores) ----
    nc = bacc.Bacc("TRN2", target_bir_lowering=False)

    d_x1i = nc.dram_tensor("x1i", [K1A, Bs], f32r, kind="ExternalInput")  # theta rows + t row
    d_x2i = nc.dram_tensor("x2i", [K1B, Bs], f32r, kind="ExternalInput")  # ctx (static)
    d_th0 = nc.dram_tensor("th0", [D, Bs], f32r, kind="ExternalInput")
    d_w1c1 = nc.dram_tensor("w1c1", [K1A, 2 * H], f32r, kind="ExternalInput")
    d_w1c2 = nc.dram_tensor("w1c2", [K1B, 2 * H], f32r, kind="ExternalInput")
    d_w2 = nc.dram_tensor("w2", [128, KC * 2 * H], f32r, kind="ExternalInput")
    d_w3 = nc.dram_tensor("w3", [128, KC * D], f32r, kind="ExternalInput")
    d_bias = nc.dram_tensor("biases", [128, 4 * MJ], f32, kind="ExternalInput")
    d_ob3 = nc.dram_tensor("onesb3", [1, Bs + D], f32r, kind="ExternalInput")
    d_out = nc.dram_tensor("out", [D, Bs], f32, kind="ExternalOutput")

    # RK4 coefficients: arg scale (for next eval's input), acc scale
    c_arg = [0.5 * dt, 0.5 * dt, dt]
    a_acc = [dt / 6.0, dt / 3.0, dt / 3.0, dt / 6.0]

    with tile.TileContext(nc) as tc:
        PS3_SHARE = int(os.environ.get("KERNEL_PS3SHARE", "0"))
        PSMM_BUFS = 8 if PS3_SHARE else 7
        with (
            tc.tile_pool(name="const", bufs=1) as cpool,
            tc.tile_pool(name="psmm", bufs=PSMM_BUFS, space="PSUM") as ps_pool,
            tc.tile_pool(name="ps3", bufs=1, space="PSUM") as ps3_pool,
            tc.tile_pool(name="sig", bufs=int(os.environ.get("KERNEL_SIGB", "10"))) as sig_pool,
            tc.tile_pool(name="hact", bufs=int(os.environ.get("KERNEL_HB", "20"))) as h_pool,
            tc.tile_pool(name="accp", bufs=int(os.environ.get("KERNEL_AB", "6"))) as acc_pool,
        ):
            tw1c1 = cpool.tile([K1A, 2 * H], f32r)
            tw1c2 = cpool.tile([K1B, 2 * H], f32r)
            tw2 = cpool.tile([128, KC * 2 * H], f32r)
            tw3 = cpool.tile([128, KC * D], f32r)
            tbias = cpool.tile([128, 4 * MJ], f32)
            tb1a = tbias[:, 0 * MJ : 1 * MJ]
            tb1b = tbias[:, 1 * MJ : 2 * MJ]
            tb2a = tbias[:, 2 * MJ : 3 * MJ]
            tb2b = tbias[:, 3 * MJ : 4 * MJ]
            tob3 = cpool.tile([1, Bs + D], f32r)
            tones = tob3[:, 0:Bs]
            tb3 = tob3[:, Bs : Bs + D]
            tx1 = cpool.tile([K1A, Bs], f32r)   # rows: [theta(32) | t(1)]  (dynamic)
            tx2 = cpool.tile([K1B, Bs], f32r)   # ctx (static)
            tth0 = cpool.tile([D, Bs], f32r)    # initial theta

            # layer-1-critical tensors first so eval 0 can start while
            # w2/w3 still stream
            nc.sync.dma_start(tx2[:], d_x2i[:])
            nc.sync.dma_start(tw1c2[:], d_w1c2[:])
            nc.sync.dma_start(tx1[:], d_x1i[:])
            nc.sync.dma_start(tw1c1[:], d_w1c1[:])
            nc.sync.dma_start(tbias[:], d_bias[:])
            nc.sync.dma_start(tth0[:], d_th0[:])
            nc.sync.dma_start(tw2[:], d_w2[:])
            nc.sync.dma_start(tw3[:], d_w3[:])
            nc.sync.dma_start(tob3[:], d_ob3[:])

            def mm(out_ap, lhsT_ap, rhs_ap, start, stop):
                nc.tensor.matmul(out_ap, lhsT_ap, rhs_ap, start=start, stop=stop)

            th_cur = tth0       # theta at start of current step
            t_written = 0.0     # t-row was preloaded with 0

            def issue_l1ctx():
                # static context contribution for the NEXT eval's layer 1 --
                # issued early so PE has work during the RK4 latency chain.
                # One accumulation group per PSUM bank: only the first MM may
                # carry start=True (a second start would zero the whole bank).
                tiles = []
                for j in range(MJ):
                    ps = ps_pool.tile([128, 2 * Bs], f32, tag="psmm")
                    for half, mj in ((1, j + MJ), (0, j)):
                        dst = ps[:, half * Bs : (half + 1) * Bs]
                        msl = slice(mj * 128, (mj + 1) * 128)
                        mm(dst, tw1c2[:, msl], tx2[:],
                           start=(half == 1), stop=False)
                    tiles.append(ps)
                return tiles

            SCHED = int(os.environ.get("KERNEL_SCHED", "3"))
            SPLITP = int(os.environ.get("KERNEL_SPLITPOOLS", "1"))
            SIG1T, SIG2T = ("sig1", "sig2") if SPLITP else ("sig", "sig")
            H1T, H2T = ("h1t", "h2t") if SPLITP else ("hact", "hact")
            if SCHED >= 2 or SCHED == 3:
                ps1 = issue_l1ctx()

            TOFF = (0.0, 0.5, 0.5, 1.0)
            for s in range(steps):
                for e in range(4):
                    TMEMSET_MODE = int(os.environ.get("KERNEL_TMEMSET", "0")) if SCHED == 3 else 0
                    TMEMSET_TAIL = TMEMSET_MODE == 1
                    TMEMSET_POST = TMEMSET_MODE == 2
                    TMEMSET_POOL_TAIL = TMEMSET_MODE == 3
                    t_val = (s + TOFF[e]) * dt
                    if TMEMSET_MODE == 0 and t_val != t_written:
                        nc.gpsimd.memset(tx1[D : D + 1, :].bitcast(f32), float(t_val))
                        t_written = t_val

                    last_eval = (s == steps - 1) and (e == 3)

                    if SCHED == 3:
                        # v1.5 structure, ctx MMs pre-issued (ps1 tiles);
                        # group stop goes on the LAST MM of the bank (a-half)
                        h1 = []
                        for j in range(MJ):
                            ps = ps1[j]
                            for half, mj in ((1, j + MJ), (0, j)):
                                dst = ps[:, half * Bs : (half + 1) * Bs]
                                msl = slice(mj * 128, (mj + 1) * 128)
                                mm(dst, tw1c1[:, msl], tx1[:], start=False,
                                   stop=(half == 0))
                            sg = sig_pool.tile([128, Bs], f32, tag=SIG1T)
                            nc.scalar.activation(
                                sg[:], ps[:, Bs : 2 * Bs], SIGMOID,
                                bias=tb1b[:, j : j + 1]
                            )
                            ht = h_pool.tile([128, Bs], f32r, tag=H1T)
                            nc.vector.scalar_tensor_tensor(
                                ht[:], ps[:, 0:Bs], tb1a[:, j : j + 1], sg[:],
                                ALU.add, ALU.mult,
                            )
                            h1.append(ht)
                        h2 = []
                        for j in range(MJ):
                            ps = ps_pool.tile([128, 2 * Bs], f32, tag="psmm")
                            # b-half group first so the sigmoid overlaps the
                            # a-half matmuls (keeps ACT off the eval tail)
                            dstb = ps[:, Bs : 2 * Bs]
                            for kc in range(KC):
                                csl = slice(kc * 2 * H + (j + MJ) * 128,
                                            kc * 2 * H + (j + MJ + 1) * 128)
                                mm(dstb, tw2[:, csl], h1[kc][:],
                                   start=(kc == 0), stop=(kc == KC - 1))
                            sg = sig_pool.tile([128, Bs], f32, tag=SIG2T)
                            nc.scalar.activation(
                                sg[:], dstb, SIGMOID, bias=tb2b[:, j : j + 1]
                            )
                            dsta = ps[:, 0:Bs]
                            for kc in range(KC):
                                csl = slice(kc * 2 * H + j * 128,
                                            kc * 2 * H + (j + 1) * 128)
                                mm(dsta, tw2[:, csl], h1[kc][:],
                                   start=(kc == 0), stop=(kc == KC - 1))
                            ht = h_pool.tile([128, Bs], f32r, tag=H2T)
                            nc.vector.scalar_tensor_tensor(
                                ht[:], dsta, tb2a[:, j : j + 1], sg[:],
                                ALU.add, ALU.mult,
                            )
                            h2.append(ht)
                        if not last_eval:
                            ps1_next = issue_l1ctx()
                    elif SCHED == 0:
                        # v1.5: per-j, both halves, GLU immediately
                        h1 = []
                        for j in range(MJ):
                            ps = ps_pool.tile([128, 2 * Bs], f32, tag="psmm")
                            for half, mj in ((0, j), (1, j + MJ)):
                                dst = ps[:, half * Bs : (half + 1) * Bs]
                                msl = slice(mj * 128, (mj + 1) * 128)
                                mm(dst, tw1c2[:, msl], tx2[:], start=True, stop=False)
                                mm(dst, tw1c1[:, msl], tx1[:], start=False, stop=True)
                            sg = sig_pool.tile([128, Bs], f32, tag="sig")
                            nc.scalar.activation(
                                sg[:], ps[:, Bs : 2 * Bs], SIGMOID,
                                bias=tb1b[:, j : j + 1]
                            )
                            ht = h_pool.tile([128, Bs], f32r, tag="hact")
                            nc.vector.scalar_tensor_tensor(
                                ht[:], ps[:, 0:Bs], tb1a[:, j : j + 1], sg[:],
                                ALU.add, ALU.mult,
                            )
                            h1.append(ht)
                        h2 = []
                        for j in range(MJ):
                            ps = ps_pool.tile([128, 2 * Bs], f32, tag="psmm")
                            for half, mj in ((0, j), (1, j + MJ)):
                                dst = ps[:, half * Bs : (half + 1) * Bs]
                                for kc in range(KC):
                                    csl = slice(kc * 2 * H + mj * 128,
                                                kc * 2 * H + (mj + 1) * 128)
                                    mm(dst, tw2[:, csl], h1[kc][:],
                                       start=(kc == 0), stop=(kc == KC - 1))
                            sg = sig_pool.tile([128, Bs], f32, tag="sig")
                            nc.scalar.activation(
                                sg[:], ps[:, Bs : 2 * Bs], SIGMOID,
                                bias=tb2b[:, j : j + 1]
                            )
                            ht = h_pool.tile([128, Bs], f32r, tag="hact")
                            nc.vector.scalar_tensor_tensor(
                                ht[:], ps[:, 0:Bs], tb2a[:, j : j + 1], sg[:],
                                ALU.add, ALU.mult,
                            )
                            h2.append(ht)
                    else:
                        if SCHED < 2:
                            ps1 = issue_l1ctx()
                        sg1 = []
                        for j in range(MJ):
                            dst = ps1[j][:, Bs : 2 * Bs]
                            msl = slice((j + MJ) * 128, (j + MJ + 1) * 128)
                            mm(dst, tw1c1[:, msl], tx1[:], start=False, stop=True)
                            sg = sig_pool.tile([128, Bs], f32, tag="sig")
                            nc.scalar.activation(
                                sg[:], dst, SIGMOID, bias=tb1b[:, j : j + 1]
                            )
                            sg1.append(sg)
                        h1 = []
                        for j in range(MJ):
                            dst = ps1[j][:, 0:Bs]
                            msl = slice(j * 128, (j + 1) * 128)
                            mm(dst, tw1c1[:, msl], tx1[:], start=False, stop=True)
                            ht = h_pool.tile([128, Bs], f32r, tag="hact")
                            nc.vector.scalar_tensor_tensor(
                                ht[:], dst, tb1a[:, j : j + 1], sg1[j][:],
                                ALU.add, ALU.mult,
                            )
                            h1.append(ht)
                        ps2 = []
                        for j in range(MJ):
                            ps2j = ps_pool.tile([128, 2 * Bs], f32, tag="psmm")
                            ps2.append(ps2j)
                        for kc in range(KC - 1):
                            for j in range(MJ):
                                csl = slice(kc * 2 * H + (j + MJ) * 128,
                                            kc * 2 * H + (j + MJ + 1) * 128)
                                mm(ps2[j][:, Bs : 2 * Bs], tw2[:, csl], h1[kc][:],
                                   start=(kc == 0), stop=False)
                            for j in range(MJ):
                                csl = slice(kc * 2 * H + j * 128,
                                            kc * 2 * H + (j + 1) * 128)
                                mm(ps2[j][:, 0:Bs], tw2[:, csl], h1[kc][:],
                                   start=(kc == 0), stop=False)
                        kc = KC - 1
                        sg2 = []
                        for j in range(MJ):
                            csl = slice(kc * 2 * H + (j + MJ) * 128,
                                        kc * 2 * H + (j + MJ + 1) * 128)
                            dst = ps2[j][:, Bs : 2 * Bs]
                            mm(dst, tw2[:, csl], h1[kc][:], start=False, stop=True)
                            sg = sig_pool.tile([128, Bs], f32, tag="sig")
                            nc.scalar.activation(
                                sg[:], dst, SIGMOID, bias=tb2b[:, j : j + 1]
                            )
                            sg2.append(sg)
                        h2 = []
                        for j in range(MJ):
                            csl = slice(kc * 2 * H + j * 128,
                                        kc * 2 * H + (j + 1) * 128)
                            dst = ps2[j][:, 0:Bs]
                            mm(dst, tw2[:, csl], h1[kc][:], start=False, stop=True)
                            ht = h_pool.tile([128, Bs], f32r, tag="hact")
                            nc.vector.scalar_tensor_tensor(
                                ht[:], dst, tb2a[:, j : j + 1], sg2[j][:],
                                ALU.add, ALU.mult,
                            )
                            h2.append(ht)
                        if SCHED >= 2 and not last_eval:
                            ps1_next = issue_l1ctx()

                    # ---- layer 3: k = h2 @ W3 (+ b3) in PSUM ----
                    if PS3_SHARE:
                        ps3full = ps_pool.tile([128, 2 * Bs], f32, tag="psmm")
                        ps3 = ps3full[0:D, 0:Bs]
                    else:
                        ps3 = ps3_pool.tile([D, Bs], f32, tag="ps3")
                    for kc in range(KC):
                        mm(ps3[:], tw3[:, kc * D : (kc + 1) * D], h2[kc][:],
                           start=(kc == 0), stop=(kc == KC - 1 and b3_is_zero))
                    if not b3_is_zero:
                        mm(ps3[:], tb3[:], tones[:], start=False, stop=True)

                    # ---- RK4 bookkeeping ----
                    if TMEMSET_POOL_TAIL and not last_eval:
                        nxt_s, nxt_e = (s, e + 1) if e < 3 else (s + 1, 0)
                        nxt_t = (nxt_s + TOFF[nxt_e]) * dt
                        if nxt_t != t_written:
                            nc.gpsimd.memset(
                                tx1[D : D + 1, :].bitcast(f32), float(nxt_t)
                            )
                            t_written = nxt_t
                    if TMEMSET_TAIL and not last_eval:
                        # write the NEXT eval's t-row on DVE (same engine as
                        # the arg STT -> no extra cross-engine hop on the
                        # arg -> layer-1 chain)
                        nxt_s, nxt_e = (s, e + 1) if e < 3 else (s + 1, 0)
                        nxt_t = (nxt_s + TOFF[nxt_e]) * dt
                        if nxt_t != t_written:
                            nc.vector.memset(
                                tx1[D : D + 1, :].bitcast(f32), float(nxt_t)
                            )
                            t_written = nxt_t
                    if e < 3:
                        nc.vector.scalar_tensor_tensor(
                            tx1[0:D, :], ps3[:], float(c_arg[e]), th_cur[:],
                            ALU.mult, ALU.add,
                        )
                    if TMEMSET_POST and not last_eval:
                        nxt_s, nxt_e = (s, e + 1) if e < 3 else (s + 1, 0)
                        nxt_t = (nxt_s + TOFF[nxt_e]) * dt
                        if nxt_t != t_written:
                            nc.vector.memset(
                                tx1[D : D + 1, :].bitcast(f32), float(nxt_t)
                            )
                            t_written = nxt_t
                    base = th_cur if e == 0 else acc_prev
                    if e == 3 and s != steps - 1:
                        # theta_{s+1} goes straight into the matmul input tile
                        # (keeps the Pool copy off the critical chain)...
                        nc.vector.scalar_tensor_tensor(
                            tx1[0:D, :], ps3[:], float(a_acc[e]), base[:],
                            ALU.mult, ALU.add,
                        )
                    acc_new = acc_pool.tile([D, Bs], f32, tag="accp")
                    # ...and also into its own tile (used as th_cur next step)
                    nc.vector.scalar_tensor_tensor(
                        acc_new[:], ps3[:], float(a_acc[e]), base[:],
                        ALU.mult, ALU.add,
                    )
                    acc_prev = acc_new
                    if SCHED >= 2 and not last_eval:
                        ps1 = ps1_next

                th_cur = acc_prev  # theta_{s+1}

            nc.sync.dma_start(d_out[:], th_cur[:])

    # ---- per-core input maps ----
    in_maps = []
    for c in range(N_CORES):
        sl = slice(c * Bs, (c + 1) * Bs)
        th_T = np.ascontiguousarray(np.asarray(theta0[sl], np.float32).T)
        ctx_T = np.ascontiguousarray(np.asarray(context[sl], np.float32).T)
        x1i = np.concatenate([th_T, np.zeros((1, Bs), np.float32)], axis=0)
        in_maps.append(
            {
                "x1i": np.ascontiguousarray(x1i),
                "x2i": ctx_T,
                "th0": th_T,
                "w1c1": w1c1_h,
                "w1c2": w1c2_h,
                "w2": w2_h,
                "w3": w3_h,
                "biases": bias_h,
                "onesb3": onesb3_h,
            }
        )

    return nc, in_maps


def _build_and_run(theta0, context, W1, b1, W2, b2, W3, b3, n_steps):
    from concourse.bass_utils import run_bass_kernel_spmd

    nc, in_maps = _build_program(theta0, context, W1, b1, W2, b2, W3, b3, n_steps)
    nc.finalize()  # Bacc: split multi-sem waits + allocate registers
    res = run_bass_kernel_spmd(
        nc,
        in_maps,
        core_ids=list(range(N_CORES)),
        trace=bool(int(os.environ.get("KERNEL_TRACE", "0"))),
    )
    _build_and_run.last_results = res

    out = np.concatenate([r["out"].T for r in res.results], axis=0)
    return np.ascontiguousarray(out.astype(np.float32))


def kernel(theta0, context, W1, b1, W2, b2, W3, b3, n_steps):
    return _build_and_run(
        np.asarray(theta0), np.asarray(context), W1, b1, W2, b2, W3, b3, n_steps
    )



# revision 28
# speedup vs baseline: 1.5645x; 1.5645x over previous
"""CCNF RK4 sampling kernel for 8 Trainium2 NeuronCores. v3

Data-parallel: batch 2048 -> 256 per core -> S interleaved streams,
weights replicated. Per eval:
  L1/L3 matmuls in bf16, L2 in fp8e4m3 with DoubleRow (2 K-tiles per MM).
  GLU halves quad-packed into single PSUM banks -> one ACT sigmoid + one
  DVE STT per layer per stream.
RK4 state lives in PSUM: a per-stream accumulator bank holds
theta0 + sum(alpha_e * k_e) across ALL steps (one open accumulation
group); the eval argument theta + c_e*k_e is built inside the k matmul
group (identity-matmul adds theta, c_e folded into W3 copies), so the
only RK4 elementwise work is one PSUM->SBUF copy per eval.
b1 and the t-row are folded into the L1 matmul via extra x rows.
"""

import os

import numpy as np

N_CORES = 8


def _split_sizes(total, parts):
    base = total // parts
    rem = total - base * parts
    return [base + (1 if i < rem else 0) for i in range(parts)]


def _build_program(theta0, context, W1, b1, W2, b2, W3, b3, n_steps):
    import ml_dtypes
    import concourse.bass as bass
    import concourse.mybir as mybir
    import concourse.tile as tile
    from concourse import bacc

    f32 = mybir.dt.float32
    bf16 = mybir.dt.bfloat16
    f8 = mybir.dt.float8e4
    ALU = mybir.AluOpType
    SIGMOID = mybir.ActivationFunctionType.Sigmoid
    COPY = mybir.ActivationFunctionType.Copy
    DR = mybir.MatmulPerfMode.DoubleRow

    np_bf16 = ml_dtypes.bfloat16
    np_f8 = ml_dtypes.float8_e4m3

    B, D = theta0.shape          # 2048, 32
    C = context.shape[1]         # 128
    IN, H2 = W1.shape            # 161, 1024
    H = W2.shape[0]              # 512
    assert H2 == 2 * H and W2.shape[1] == 2 * H and W3.shape == (H, D)
    assert IN == D + 1 + C
    assert B % N_CORES == 0
    Bs = B // N_CORES            # 256 per core
    NS = int(os.environ.get("KERNEL_STREAMS", "3"))
    ns = _split_sizes(Bs, NS)    # per-stream batch sizes
    off = [sum(ns[:i]) for i in range(NS)]
    steps = int(n_steps)
    dt = 1.0 / steps

    MJ = H // 128                # 4 output chunks per GLU half
    KC = H // 128                # 4 K-chunks for layer 2/3
    KP = KC // 2                 # 2 DoubleRow K-pairs
    S2 = 16.0                    # W2 fp8 scale
    K1 = D + 2                   # theta(32) + t(1) + ones(1) = 34

    c_arg = [0.5 * dt, 0.5 * dt, dt]
    a_acc = [dt / 6.0, dt / 3.0, dt / 3.0, dt / 6.0]
    TOFF = (0.0, 0.5, 0.5, 1.0)

    # ---- host-side layout prep (shared across cores) ----
    W1 = np.asarray(W1, np.float32)
    W3 = np.asarray(W3, np.float32)
    b1 = np.asarray(b1, np.float32)
    b2 = np.asarray(b2, np.float32)
    b3 = np.asarray(b3, np.float32)
    b2_is_zero = not np.any(b2)
    b3_is_zero = not np.any(b3)

    # x1 rows: theta(0:32) | t(32) | ones(33); matching W1 rows + b1 row
    w1t_h = np.ascontiguousarray(
        np.concatenate([W1[: D + 1], b1.reshape(1, 2 * H)], axis=0).astype(np_bf16)
    )                                                           # [34, 1024]
    w1c_h = np.ascontiguousarray(W1[D + 1 :].astype(np_bf16))   # [128, 1024]
    # [512,1024] -> [128, 4, 1024]: w2dr[p, ks, m] = S2*W2[ks*128+p, m]
    L2M_ = os.environ.get("KERNEL_L2M", "dr")
    if L2M_ == "bf":
        S2 = 1.0
    w2_np = np_bf16 if L2M_ == "bf" else np_f8
    w2_h = np.ascontiguousarray(
        (np.asarray(W2, np.float32) * S2)
        .reshape(KC, 128, 2 * H)
        .transpose(1, 0, 2)
        .astype(w2_np)
    )                                                           # [128, 4, 1024]

    # [512,32] -> [128, 4*32] with scale: w3[p, kc*32+d] = s*W3[kc*128+p, d]
    def w3_scaled(s):
        return np.ascontiguousarray(
            (W3 * s).reshape(KC, 128, D).transpose(1, 0, 2).reshape(128, KC * D)
            .astype(np_bf16)
        )

    # k-group scales: c_arg; acc-group scales: a_acc (dt/2, dt, dt/6, dt/3)
    L3M_ = os.environ.get("KERNEL_L3M", "bf")
    S3H_ = 4.0
    S3W = 16.0
    if L3M_ == "dr":
        # fp8 weights must stay in normal range: quantize W3*S3W and put
        # the small RK4 scales into identity entries / copy scales.
        # k-bank   = (S3W/c)*theta + S3W*W3^T h2  -> copy scale c/S3W
        # acc-bank = F*theta + F*sum(alpha*W3^T h2), F = 3*S3W/dt
        def w3f8(s):
            return np.ascontiguousarray(
                (W3 * (s / S3H_)).reshape(KC, 128, D).transpose(1, 0, 2)
                .reshape(128, KC * D).astype(np_f8)
            )
        w3c2_h = w3f8(S3W)            # k MMs (both c variants)
        w3c1_h = w3f8(S3W)
        w3a6_h = w3f8(S3W / 2.0)      # acc alpha=dt/6
        w3a3_h = w3f8(S3W)            # acc alpha=dt/3
        F_ACC = 3.0 * S3W / dt        # 768 for dt=1/16
        ID_K = [S3W / (0.5 * dt), S3W / (0.5 * dt), S3W / dt]  # per c_arg
        CP_ARG = [1.0 / v for v in ID_K]
        CP_ACC = 1.0 / F_ACC
    else:
        w3c2_h = w3_scaled(0.5 * dt)   # e0, e1 arg
        w3c1_h = w3_scaled(dt)         # e2 arg
        w3a6_h = w3_scaled(dt / 6.0)   # e0, e3 acc
        w3a3_h = w3_scaled(dt / 3.0)   # e1, e2 acc
        F_ACC = 1.0
        ID_K = [1.0, 1.0, 1.0]
        CP_ARG = [1.0, 1.0, 1.0]
        CP_ACC = 1.0
    # b3 enters k unscaled via the ones row of x1 through the identity-MM
    # trick is not available (k = theta + c*(W3^T h2 + b3)); handle b3 by a
    # rank-1 MM with the scaled b3 row when nonzero.
    b2_h = np.ascontiguousarray((b2 * S2).reshape(1, 2 * H).astype(np_bf16))
    id_bf_h = np.ascontiguousarray(np.concatenate([
        np.eye(D, dtype=np.float32) * ID_K[0],
        np.eye(D, dtype=np.float32) * ID_K[2],
        np.eye(D, dtype=np.float32) * F_ACC,
    ], axis=1).astype(np_bf16))                       # [D, 3D]
    id_f32_h = np.ascontiguousarray(np.eye(D, dtype=np.float32) * F_ACC)

    # ---- build the bass program (same program on all 8 cores) ----
    nc = bacc.Bacc("TRN2", target_bir_lowering=False)

    d_x1i = nc.dram_tensor("x1i", [K1, Bs], bf16, kind="ExternalInput")
    d_ctx = nc.dram_tensor("ctx", [C, Bs], bf16, kind="ExternalInput")
    d_th0 = nc.dram_tensor("th0", [D, Bs], f32, kind="ExternalInput")
    d_w1t = nc.dram_tensor("w1t", [K1, 2 * H], bf16, kind="ExternalInput")
    d_w1c = nc.dram_tensor("w1c", [C, 2 * H], bf16, kind="ExternalInput")
    w2dt = bf16 if L2M_ == "bf" else f8
    d_w2 = nc.dram_tensor("w2", [128, KC, 2 * H], w2dt, kind="ExternalInput")
    w3dt = f8 if L3M_ == "dr" else bf16
    d_w3x = nc.dram_tensor("w3x", [128, 4 * KC * D], w3dt, kind="ExternalInput")
    d_b2 = nc.dram_tensor("b2s", [1, 2 * H], bf16, kind="ExternalInput")
    d_b3x = nc.dram_tensor("b3x", [2, D], bf16, kind="ExternalInput")
    d_idb = nc.dram_tensor("idb", [D, 3 * D], bf16, kind="ExternalInput")
    d_idf = nc.dram_tensor("idf", [D, D], f32, kind="ExternalInput")
    d_out = nc.dram_tensor("out", [D, Bs], f32, kind="ExternalOutput")

    CPENG = os.environ.get("KERNEL_CPENG", "act")  # arg/x1s copy: act|dve|alt
    L2M = os.environ.get("KERNEL_L2M", "dr")  # dr (fp8 DoubleRow) | f8 | bf
    SEEDBF = int(os.environ.get("KERNEL_SEEDBF", "0"))
    L3M = os.environ.get("KERNEL_L3M", "bf")  # bf | dr (fp8 DoubleRow)
    S3H = 4.0  # h2 fp8 pre-scale (keeps h2 in fp8 normal range)
    PSB = int(os.environ.get("KERNEL_PSB", "5"))
    PSKB = int(os.environ.get("KERNEL_PSKB", "1"))
    ACCB = int(os.environ.get("KERNEL_ACCB", "2"))
    SIGB = int(os.environ.get("KERNEL_SIGB", "12"))
    SPLIT = int(os.environ.get("KERNEL_SPLIT", "0"))  # split GLU ops per k-pair
    HB = int(os.environ.get("KERNEL_HB", "8"))
    BANKSPLIT = int(os.environ.get("KERNEL_BANKSPLIT", "0"))

    with tile.TileContext(nc) as tc:
        with (
            tc.tile_pool(name="const", bufs=1) as cpool,
            tc.tile_pool(name="ps", bufs=PSB, space="PSUM") as ps_pool,
            tc.tile_pool(name="psk", bufs=PSKB, space="PSUM") as psk_pool,
            tc.tile_pool(name="acc", bufs=ACCB, space="PSUM") as acc_pool,
            tc.tile_pool(name="sig", bufs=SIGB) as sig_pool,
            tc.tile_pool(name="hp", bufs=HB) as h_pool,
        ):
            tw1t = cpool.tile([K1, 2 * H], bf16)
            tw1c = cpool.tile([C, 2 * H], bf16)
            tw2 = cpool.tile([128, KC, 2 * H], w2dt)
            tw3x = cpool.tile([128, 4, KC, D], w3dt)
            tw3c = [tw3x[:, 0], tw3x[:, 1]]   # c=dt/2 (e0,e1), dt (e2)
            tw3a = [tw3x[:, 2], tw3x[:, 3]]   # a=dt/6 (e0,e3), dt/3
            tb2 = cpool.tile([1, 2 * H], bf16)
            tb3x = cpool.tile([2, D], bf16)  # rows: b3*dt... see b3 handling
            tidb = cpool.tile([D, 3 * D], bf16)
            tid_k = [tidb[:, 0:D], tidb[:, 0:D], tidb[:, D : 2 * D]]
            tid_acc = tidb[:, 2 * D : 3 * D]
            tidf = cpool.tile([D, D], f32)
            tx1s = [cpool.tile([K1, ns[i]], bf16, name=f"tx1s{i}") for i in range(NS)]
            tx1a = [cpool.tile([K1, ns[i]], bf16, name=f"tx1a{i}") for i in range(NS)]
            tctx = [cpool.tile([C, ns[i]], bf16, name=f"tctx{i}") for i in range(NS)]
            tout = [cpool.tile([D, ns[i]], f32, name=f"tout{i}") for i in range(NS)]
            # fp32 theta state, rotated per step; updated from the acc bank
            tthf = [[cpool.tile([D, ns[i]], f32, name=f"tthf{i}_{j}")
                     for j in range(2)] for i in range(NS)]

            # layer-1-critical tensors first so eval 0 starts early
            for si in range(NS):
                sl = slice(off[si], off[si] + ns[si])
                nc.sync.dma_start(tctx[si][:], d_ctx[:, sl])
                nc.sync.dma_start(tx1s[si][:], d_x1i[:, sl])
            nc.sync.dma_start(tw1c[:], d_w1c[:])
            nc.sync.dma_start(tw1t[:], d_w1t[:])
            for si in range(NS):
                sl = slice(off[si], off[si] + ns[si])
                nc.sync.dma_start(tthf[si][0][:], d_th0[:, sl])
            nc.sync.dma_start(tw2[:], d_w2[:])
            nc.sync.dma_start(tw3x[:], d_w3x[:])
            nc.sync.dma_start(tb2[:], d_b2[:])
            nc.sync.dma_start(tb3x[:], d_b3x[:])
            nc.sync.dma_start(tidb[:], d_idb[:])
            nc.sync.dma_start(tidf[:], d_idf[:])

            # ones row of the arg tile (state tile ships with ones from host;
            # memset can't start at partition 33 -- not 32-aligned)
            for si in range(NS):
                sl = slice(off[si], off[si] + ns[si])
                nc.sync.dma_start(tx1a[si][D + 1 : D + 2, :],
                                  d_x1i[D + 1 : D + 2, sl])

            mm = nc.tensor.matmul

            t_written = [0.0] * NS  # t-row of x1a
            ncopy = [0]

            def copy_psum(dst, src):
                use_act = (CPENG == "act") or (CPENG == "alt" and ncopy[0] % 2 == 0)
                ncopy[0] += 1
                if use_act:
                    nc.scalar.activation(dst, src, COPY, scale=scale)
                else:
                    if scale == 1.0:
                        nc.vector.tensor_scalar_add(dst, src, 0.0)
                    else:
                        nc.vector.tensor_scalar_mul(dst, src, scale)

            pacc = [None] * NS

            def stream_prog(si):
                N = ns[si]
                for s in range(steps):
                    for e in range(4):
                        last_eval = (s == steps - 1) and (e == 3)
                        if e == 0:
                            pacc[si] = acc_pool.tile([D, 512], f32, tag="acc",
                                                     name="pacc")
                            if SEEDBF and s > 0:
                                mm(pacc[si][:, 0:N], tid_acc,
                                   tx1s[si][0:D, :], start=True, stop=False)
                            else:
                                mm(pacc[si][:, 0:N], tidf[:],
                                   tthf[si][s % 2][:], start=True, stop=False)
                        x_in = tx1s[si] if e == 0 else tx1a[si]

                        # ---- stage 0: layer 1 MMs (2 quad banks) ----
                        psa = ps_pool.tile([128, MJ * N], f32, tag="ps", name="psa")
                        psb = ps_pool.tile([128, MJ * N], f32, tag="ps", name="psb")
                        for bank, hoff in ((psb, H), (psa, 0)):
                            for m in range(MJ):
                                msl = slice(hoff + m * 128, hoff + (m + 1) * 128)
                                mm(bank[:, m * N : (m + 1) * N],
                                   tw1c[:, msl], tctx[si][:],
                                   start=(m == 0), stop=False)
                            for m in range(MJ):
                                msl = slice(hoff + m * 128, hoff + (m + 1) * 128)
                                mm(bank[:, m * N : (m + 1) * N],
                                   tw1t[:, msl], x_in[:],
                                   start=False, stop=(m == MJ - 1))
                            if BANKSPLIT:
                                yield
                        if not BANKSPLIT:
                            yield

                        # ---- stage 1: sig1 + h1 ----
                        Np = (N + 15) // 16 * 16
                        sg1 = sig_pool.tile([128, MJ * N], bf16, tag="sig",
                                            name="sg")
                        h1f = h_pool.tile([128, KC, Np], w2dt, tag="h1",
                                          name="h1t")
                        h1 = h1f[:, :, 0:N]
                        if SPLIT:
                            half = 2 * N
                            for p in range(2):
                                hs = slice(p * half, (p + 1) * half)
                                nc.scalar.activation(
                                    sg1[:, hs], psb[:, hs], SIGMOID)
                            for p in range(2):
                                hs = slice(p * half, (p + 1) * half)
                                ksl = slice(2 * p, 2 * p + 2)
                                nc.vector.scalar_tensor_tensor(
                                    h1[:, ksl, :], psa[:, hs], 1.0,
                                    sg1[:, hs], ALU.mult, ALU.mult)
                        else:
                            nc.scalar.activation(sg1[:], psb[:], SIGMOID)
                            nc.vector.scalar_tensor_tensor(
                                h1[:], psa[:], 1.0, sg1[:],
                                ALU.mult, ALU.mult,
                            )
                        yield

                        # ---- stage 2: layer 2 MMs ----
                        psA = ps_pool.tile([128, MJ * N], f32, tag="ps", name="psA")
                        psB = ps_pool.tile([128, MJ * N], f32, tag="ps", name="psB")
                        for bank, hoff in ((psB, H), (psA, 0)):
                            first = True
                            if not b2_is_zero:
                                for m in range(MJ):
                                    msl = slice(hoff + m * 128, hoff + (m + 1) * 128)
                                    mm(bank[:, m * N : (m + 1) * N],
                                       tb2[:, msl], x_in[D + 1 : D + 2, :],
                                       start=first, stop=False)
                                    first = False
                            for m in range(MJ):
                                msl = slice(hoff + m * 128, hoff + (m + 1) * 128)
                                if L2M == "dr":
                                    for kp in range(KP):
                                        ksl = slice(kp * 2, (kp + 1) * 2)
                                        mm(bank[:, m * N : (m + 1) * N],
                                           tw2[:, ksl, msl], h1[:, ksl, :],
                                           start=(first and m == 0 and kp == 0),
                                           stop=(m == MJ - 1 and kp == KP - 1),
                                           perf_mode=DR)
                                else:
                                    for kc in range(KC):
                                        mm(bank[:, m * N : (m + 1) * N],
                                           tw2[:, kc, msl], h1[:, kc, :],
                                           start=(first and m == 0 and kc == 0),
                                           stop=(m == MJ - 1 and kc == KC - 1))
                            if BANKSPLIT:
                                yield
                        if not BANKSPLIT:
                            yield

                        # ---- stage 3: sig2 + h2 ----
                        sg2 = sig_pool.tile([128, MJ * N], bf16, tag="sig",
                                            name="sg2")
                        h2f = h_pool.tile([128, KC, Np],
                                          f8 if L3M == "dr" else bf16,
                                          tag="h2", name="h2t")
                        h2 = h2f[:, :, 0:N]
                        if SPLIT:
                            half = 2 * N
                            for p in range(2):
                                hs = slice(p * half, (p + 1) * half)
                                nc.scalar.activation(
                                    sg2[:, hs], psB[:, hs], SIGMOID,
                                    scale=1.0 / S2)
                            for p in range(2):
                                hs = slice(p * half, (p + 1) * half)
                                ksl = slice(2 * p, 2 * p + 2)
                                nc.vector.scalar_tensor_tensor(
                                    h2[:, ksl, :], psA[:, hs],
                                    (S3H if L3M == "dr" else 1.0) / S2,
                                    sg2[:, hs], ALU.mult, ALU.mult)
                        else:
                            nc.scalar.activation(sg2[:], psB[:], SIGMOID,
                                                 scale=1.0 / S2)
                            nc.vector.scalar_tensor_tensor(
                                h2[:], psA[:],
                                (S3H if L3M == "dr" else 1.0) / S2, sg2[:],
                                ALU.mult, ALU.mult,
                            )
                        yield

                        # ---- stage 4: layer 3 + RK4 copies ----
                        w3a_t = tw3a[0] if e in (0, 3) else tw3a[1]
                        if e < 3:
                            # k-bank: theta_s + c_e * (W3^T h2 + b3); the
                            # next eval's arg is a plain copy of it
                            w3c_t = tw3c[0] if e < 2 else tw3c[1]
                            b3r = tb3x[0:1] if e < 2 else tb3x[1:2]
                            pk = psk_pool.tile([D, 512], f32, tag="psk",
                                               name="pk")
                            dst = pk[:, 0:N]
                            mm(dst, tid_k[e], x_in[0:D, :],
                               start=True, stop=False)
                            if not b3_is_zero:
                                mm(dst, b3r, x_in[D + 1 : D + 2, :],
                                   start=False, stop=False)
                            if L3M == "dr":
                                for kp in range(KP):
                                    ksl = slice(kp * 2, (kp + 1) * 2)
                                    mm(dst, w3c_t[:, ksl, :], h2[:, ksl, :],
                                       start=False, stop=(kp == KP - 1),
                                       perf_mode=DR)
                            else:
                                for kc in range(KC):
                                    mm(dst, w3c_t[:, kc, :], h2[:, kc, :],
                                       start=False, stop=(kc == KC - 1))
                            copy_psum(tx1a[si][0:D, :], dst,
                                      "arg2" if e == 2 else "arg", si,
                                      scale=CP_ARG[e])
                            nxt_t = (s + TOFF[e + 1]) * dt
                            if nxt_t != t_written[si]:
                                nc.gpsimd.memset(
                                    tx1a[si][D : D + 1, :], float(nxt_t)
                                )
                                t_written[si] = nxt_t
                        # acc-bank: += a_e * (W3^T h2 + b3)
                        dst = pacc[si][:, 0:N]
                        if not b3_is_zero:
                            b3a = tb3x[0:1] if e in (0, 3) else tb3x[1:2]
                            mm(dst, b3a, x_in[D + 1 : D + 2, :],
                               start=False, stop=False)
                        if L3M == "dr":
                            for kp in range(KP):
                                ksl = slice(kp * 2, (kp + 1) * 2)
                                mm(dst, w3a_t[:, ksl, :], h2[:, ksl, :],
                                   start=False,
                                   stop=(e == 3 and kp == KP - 1),
                                   perf_mode=DR)
                        else:
                            for kc in range(KC):
                                mm(dst, w3a_t[:, kc, :], h2[:, kc, :],
                                   start=False,
                                   stop=(e == 3 and kc == KC - 1))
                        if e == 3 and not last_eval:
                            # theta_{s+1}: bf16 first (chain-critical for
                            # the next step's L1), exact fp32 after
                            copy_psum(tx1s[si][0:D, :], dst, "step", si,
                                      scale=CP_ACC)
                            nc.gpsimd.memset(
                                tx1s[si][D : D + 1, :], float((s + 1) * dt)
                            )
                            if not SEEDBF:
                                copy_psum(tthf[si][(s + 1) % 2][:], dst,
                                          "step", si, scale=CP_ACC)
                        yield

                copy_psum(tout[si][:], pacc[si][:, 0:N], "step", si,
                          scale=CP_ACC)
                sl = slice(off[si], off[si] + ns[si])
                nc.sync.dma_start(d_out[:, sl], tout[si][:])

            # drive the per-stream programs round-robin, staggered by
            # KERNEL_OFFST pipeline stages so engine demand interleaves
            OFFST = int(os.environ.get("KERNEL_OFFST", "0"))
            gens = [stream_prog(si) for si in range(NS)]
            alive = [True] * NS
            tick = 0
            ROT = int(os.environ.get("KERNEL_ROT", "0"))
            while any(alive):
                order = list(range(NS))
                if ROT:
                    r = (tick // max(ROT, 1)) % NS
                    order = order[r:] + order[:r]
                for si in order:
                    if alive[si] and tick >= si * OFFST:
                        try:
                            next(gens[si])
                        except StopIteration:
                            alive[si] = False
                tick += 1

    # b3 scaling sanity: fold c/a scales into the b3 rows we ship
    # row0: b3 * (dt/2) [e0,e1 arg] ... but acc uses different scales; we
    # shipped only 2 rows. If b3 != 0 the kernel above needs per-use scaled
    # rows; keep it correct by asserting the common case.
    if not b3_is_zero:
        raise NotImplementedError(
            "nonzero b3 needs per-scale b3 rows; add rows to d_b3x"
        )

    # ---- per-core input maps ----
    w3x_h = np.ascontiguousarray(
        np.concatenate([w3c2_h, w3c1_h, w3a6_h, w3a3_h], axis=1)
    )
    b3x_h = np.ascontiguousarray(
        np.stack([(b3 * 0.5 * dt), (b3 * dt)], axis=0).astype(np_bf16)
    )
    in_maps = []
    for c in range(N_CORES):
        sl = slice(c * Bs, (c + 1) * Bs)
        th_T = np.ascontiguousarray(np.asarray(theta0[sl], np.float32).T)
        ctx_T = np.ascontiguousarray(
            np.asarray(context[sl], np.float32).T.astype(np_bf16)
        )
        x1i = np.concatenate(
            [
                th_T.astype(np_bf16),
                np.zeros((1, Bs), np_bf16),          # t = 0
                np.ones((1, Bs), np_bf16),           # ones row
            ],
            axis=0,
        )
        in_maps.append(
            {
                "x1i": np.ascontiguousarray(x1i),
                "ctx": ctx_T,
                "th0": th_T,
                "w1t": w1t_h,
                "w1c": w1c_h,
                "w2": w2_h,
                "w3x": w3x_h,
                "b2s": b2_h,
                "b3x": b3x_h,
                "idb": id_bf_h,
                "idf": id_f32_h,
            }
        )

    return nc, in_maps


def _build_and_run(theta0, context, W1, b1, W2, b2, W3, b3, n_steps):
    from concourse.bass_utils import run_bass_kernel_spmd

    nc, in_maps = _build_program(theta0, context, W1, b1, W2, b2, W3, b3, n_steps)
    nc.finalize()  # Bacc: split multi-sem waits + allocate registers
    res = run_bass_kernel_spmd(
        nc,
        in_maps,
        core_ids=list(range(N_CORES)),
        trace=bool(int(os.environ.get("KERNEL_TRACE", "0"))),
    )
    _build_and_run.last_results = res

    out = np.concatenate([r["out"].T for r in res.results], axis=0)
    return np.ascontiguousarray(out.astype(np.float32))


def kernel(theta0, context, W1, b1, W2, b2, W3, b3, n_steps):
    return _build_and_run(
        np.asarray(theta0), np.asarray(context), W1, b1, W2, b2, W3, b3, n_steps
    )
